# revision 33
# baseline (speedup 1.0000x reference)
"""3-layer GAT (nn_GAT_64003602645176) on 8 TRN2 NeuronCores via Bass.

Sharding: nodes across 8 cores, edges partitioned by dst.  Per layer every
core holds a full AllGather'd table of rows [m_l (64 bf16) | s_l (1 f32) |
pad] (256B rows) where m_l = h_l @ w_{l+1} is the NEXT layer's message and
s_l = m_l @ a_s its source score; the per-layer linear is folded into the
table (GAT aggregation is linear in w).  Edge gathers via gpsimd.dma_gather
(int16 idx, 4 row-banks of 25088 rows, 2 SWDGE queues).  Chunk packing via
global shared-shape set-cover so all 8 cores share one column-width profile
(low padding + balanced load).  Aggregation via identity-lhsT matmuls
accumulating [ex*m | ex] into PSUM per 128-node chunk.  Self-loops are not
gathered: their contribution is added from the local stage tile.  Softmax
without max subtraction (scores are small); dummy slots gather a row whose
s is -1e5 so their weight is 0.  Per-column dst scores d are expanded from
per-chunk values with a static 0/1 matmul.  Tables are chunk-major so the
AllGather can be split into strided-output chunks overlapping compute.
"""
import sys
import numpy as np

sys.path.insert(0, "/opt/trn_rl_repo")
import ml_dtypes  # noqa: E402,F401

N, E = 100000, 1280000
IND, F = 256, 64
NEG = 0.2
NCORE = 8
NPC = 12500
P = 128
CPC = 98
SPC = CPC * P               # 12544 table rows per core
BROWS = 2 * SPC             # 25088 rows per bank
TROWS = NCORE * SPC
ESZ = 128                   # bf16 elems per row (256B)
HS_F32COL = 32
GROUP = 7
NG = CPC // GROUP
RW = 65
DMAX = 20
GCALL_COLS = 8              # dest cols per dma_gather call (1024 idxs)
NQ = 4                      # SWDGE queues
CC_CHUNKS = [5, 4, 3, 2]    # groups per AllGather chunk; each chunk gathers
                            # into a contiguous staging tensor, then a local
                            # strided DMA repacks it into the k-major table
CC_DELAY = 1                # publish a chunk this many groups after its data
                            # is ready so the in-order gpsimd queue never
                            # stalls waiting for the chunk's compute
PH0_CHUNKS = [14]           # phase 0 has no gather stream to hide behind:
                            # one full-table AllGather (no repack) is cheaper
C_LAST = CPC - 1
DUMMY_LOCAL = C_LAST * P + 127   # chunk-major local row within even core

_cache = {}


# ======================= host preprocessing =======================

def _preprocess(src, dst):
    s = np.asarray(src, np.int64) % N
    d = np.asarray(dst, np.int64) % N
    deg = np.bincount(d, minlength=N)

    # ---- node -> core: LPT on in-degree ----
    import heapq
    order = np.argsort(-deg, kind="stable")
    heap = [(0, 0, k) for k in range(NCORE)]
    heapq.heapify(heap)
    core_of = np.empty(N, np.int32)
    for n in order:
        load, cnt, k = heapq.heappop(heap)
        core_of[n] = k
        if cnt + 1 < NPC:
            heapq.heappush(heap, (load + int(deg[n]), cnt + 1, k))
    bank_of_node = core_of // 2

    eb = bank_of_node[s]
    db4 = np.zeros((N, 4), np.int64)
    for b in range(4):
        db4[:, b] = np.bincount(d[eb == b], minlength=N)

    # ---- global shared-shape set-cover packing ----
    profs = [db4[core_of == k] for k in range(NCORE)]
    nodes_of = [np.where(core_of == k)[0] for k in range(NCORE)]
    clipped = [np.minimum(p, DMAX) for p in profs]
    unassigned = [np.ones(len(p), bool) for p in profs]
    asg = [np.full(len(p), -1, np.int32) for p in profs]
    Wenv = np.zeros((CPC, 4), np.int64)
    # reserved dummy position: (box C_LAST, partition 127) kept empty;
    # box C_LAST capacity 127 on every core.
    caps = np.full(CPC, P, np.int64)

    rng_cost = (np.arange(DMAX + 1)[:, None, None, None]
                + np.arange(DMAX + 1)[None, :, None, None]
                + np.arange(DMAX + 1)[None, None, :, None]
                + np.arange(DMAX + 1)[None, None, None, :]).astype(np.float64)

    def count_tensors():
        Cs = []
        for k in range(NCORE):
            C = np.zeros((DMAX + 1,) * 4, np.int32)
            idx = clipped[k][unassigned[k]]
            np.add.at(C, (idx[:, 0], idx[:, 1], idx[:, 2], idx[:, 3]), 1)
            Cs.append(C.cumsum(0).cumsum(1).cumsum(2).cumsum(3))
        return Cs

    nbox = 0
    while nbox < CPC:
        rem_boxes = CPC - nbox
        Cs = count_tensors()
        fit_min = np.minimum.reduce(Cs)
        rem_nodes_max = max(int(u.sum()) for u in unassigned)
        need_fill = rem_nodes_max - (rem_boxes - 1) * P
        fit = np.minimum(fit_min, P)
        ratio = fit / np.maximum(rng_cost, 1)
        ratio[fit < min(P, max(need_fill, 1))] = -1
        if ratio.max() <= 0:
            ratio = np.minimum(fit_min, P) / np.maximum(rng_cost, 1)
        shape = np.array(np.unravel_index(np.argmax(ratio), ratio.shape),
                         np.int64)
        for k in range(NCORE):
            cand = np.where(unassigned[k]
                            & (clipped[k] <= shape[None, :]).all(1))[0]
            cand = cand[np.argsort(-profs[k][cand].sum(1),
                                   kind="stable")][:P]
            asg[k][cand] = nbox
            unassigned[k][cand] = False
            if len(cand):
                Wenv[nbox] = np.maximum(Wenv[nbox], profs[k][cand].max(0))
        nbox += 1

    # leftovers -> boxes with space, min growth (box C_LAST holds <=127)
    for k in range(NCORE):
        left = np.where(unassigned[k])[0]
        if not len(left):
            continue
        left = left[np.argsort(-profs[k][left].sum(1))]
        space = np.bincount(asg[k][asg[k] >= 0], minlength=CPC)
        for n_i in left:
            pv = profs[k][n_i]
            grow = np.maximum(pv[None, :] - Wenv, 0).sum(1).astype(np.float64)
            limit = np.where(np.arange(CPC) == C_LAST, P - 1, P)
            grow[space >= limit] = np.inf
            c = int(np.argmin(grow))
            asg[k][n_i] = c
            space[c] += 1
            Wenv[c] = np.maximum(Wenv[c], pv)
    # enforce reserved dummy slot: if any core filled box C_LAST to 128,
    # move one node out (should not happen due to limit above)
    for k in range(NCORE):
        mem = np.where(asg[k] == C_LAST)[0]
        assert len(mem) <= P - 1

    # ---- box -> group assignment (LPT balance on total width, 7/group) ----
    totw = Wenv.sum(1)
    order_b = np.argsort(-totw, kind="stable")
    gload = np.zeros(NG, np.int64)
    gcnt = np.zeros(NG, np.int64)
    slot_of_box = np.empty(CPC, np.int64)
    gslots = [[] for _ in range(NG)]
    for b_i in order_b:
        if b_i == C_LAST:
            continue
        g = min((g for g in range(NG) if gcnt[g] < GROUP
                 - (1 if g == NG - 1 else 0)),
                key=lambda g: (gload[g], gcnt[g]))
        gslots[g].append(b_i)
        gload[g] += totw[b_i]
        gcnt[g] += 1
    gslots[NG - 1].append(C_LAST)   # dummy box last
    # relabel groups by descending load so the tail groups are light (their
    # compute finishes fast, shrinking the last AllGather chunk's latency);
    # keep the dummy-box group last regardless.
    order_g = sorted(range(NG - 1), key=lambda g: -gload[g]) + [NG - 1]
    gslots = [gslots[g] for g in order_g]
    for g in range(NG):
        assert len(gslots[g]) == GROUP
        for j, b_i in enumerate(gslots[g]):
            slot_of_box[b_i] = g * GROUP + j
    # remap: slot c (0..97); D widths per slot
    D = np.zeros((CPC, 4), np.int64)
    D[slot_of_box] = Wenv
    new_c_last = slot_of_box[C_LAST]
    assert new_c_last == CPC - 1

    # ---- node_at [core, slot, p] ----
    node_at = np.full((NCORE, CPC, P), -1, np.int64)
    for k in range(NCORE):
        slots = slot_of_box[asg[k]]
        for c in range(CPC):
            mem = nodes_of[k][slots == c]
            mem = mem[np.argsort(-deg[mem], kind="stable")]
            node_at[k, c, :len(mem)] = mem
    assert (node_at[:, C_LAST, 127] == -1).all()

    # table row (chunk-major within core): k*SPC + c*P + p
    tbl_row = np.empty(N, np.int64)
    k_idx, c_idx, p_idx = np.nonzero(node_at >= 0)
    orig = node_at[k_idx, c_idx, p_idx]
    tbl_row[orig] = k_idx * SPC + c_idx * P + p_idx
    newid = np.empty(N, np.int64)
    newid[orig] = k_idx * SPC + c_idx * P + p_idx

    # ---- per-group column layout: (b, c, j), bank-major ----
    group_cols = np.zeros(NG, np.int64)
    bofs = np.zeros((NG, 5), np.int64)
    seg_off = np.zeros((CPC, 4), np.int64)    # column offset of (c,b) seg
    for g in range(NG):
        o = 0
        for b in range(4):
            bofs[g, b] = o
            for c in range(g * GROUP, (g + 1) * GROUP):
                seg_off[c, b] = o
                o += int(D[c, b])
        bofs[g, 4] = o
        group_cols[g] = o
    COLS_MAX = int(group_cols.max())
    gofs = np.concatenate([[0], np.cumsum(group_cols)])
    TOTCOLS = int(group_cols.sum())

    # ---- edge sort & idx grid ----
    e_order = np.lexsort((tbl_row[s], eb, d))
    ds, ss, ebs = d[e_order], s[e_order], eb[e_order]
    keys = ds * 4 + ebs
    starts = np.searchsorted(
        keys, np.arange(N, dtype=np.int64)[:, None] * 4 + np.arange(4)[None, :],
        side="left")

    idx_grid = np.full((NCORE, TOTCOLS, P), DUMMY_LOCAL, np.int64)
    for k in range(NCORE):
        for c in range(CPC):
            g = c // GROUP
            nodes = node_at[k, c]
            nv = np.where(nodes >= 0)[0]
            nn = nodes[nv]
            for b in range(4):
                J = int(D[c, b])
                if J == 0:
                    continue
                cnt = db4[nn, b]
                jj = np.arange(J)[:, None]
                has = jj < cnt[None, :]
                st = np.minimum(starts[nn, b][None, :] + jj, len(ss) - 1)
                rows = tbl_row[ss[st]] - b * BROWS
                rows = np.where(has, rows, DUMMY_LOCAL)
                g0 = gofs[g] + seg_off[c, b]
                idx_grid[k, g0:g0 + J][:, nv] = rows
    assert idx_grid.min() >= 0 and idx_grid.max() < BROWS
    idx_grid = idx_grid.astype(np.int16)

    # gather calls: per (g, b) segment split into <=GCALL_COLS columns
    calls = []
    gidx_cols_total = 0
    for g in range(NG):
        for b in range(4):
            c0, c1 = int(bofs[g, b]), int(bofs[g, b + 1])
            w0 = c0
            while w0 < c1:
                w = min(GCALL_COLS, c1 - w0)
                calls.append((g, b, w0, w, gidx_cols_total))
                gidx_cols_total += w * 8
                w0 += w
    gidx = np.zeros((NCORE, P, gidx_cols_total), np.int16)
    for k in range(NCORE):
        for (g, b, w0, w, gc0) in calls:
            flat = idx_grid[k, gofs[g] + w0: gofs[g] + w0 + w].reshape(-1)
            wrapped = flat.reshape(-1, 16).T
            gidx[k, :, gc0:gc0 + w * 8] = np.tile(wrapped, (8, 1))

    # expand matrices: [7, TOTCOLS], 1 at (c_local, col) if col in seg of c
    expand = np.zeros((GROUP, TOTCOLS), np.float32)
    for g in range(NG):
        for b in range(4):
            for ci in range(GROUP):
                c = g * GROUP + ci
                o = gofs[g] + seg_off[c, b]
                expand[ci, o:o + int(D[c, b])] = 1.0

    return dict(node_at=node_at, D=D, newid=newid, group_cols=group_cols,
                COLS_MAX=COLS_MAX, bofs=bofs, gofs=gofs, seg_off=seg_off,
                calls=calls, gidx=gidx, gidx_cols_total=gidx_cols_total,
                expand=expand, TOTCOLS=TOTCOLS)


# ======================= device program =======================

def _build_nc(meta):
    import concourse.bass as bass  # noqa: F401
    import concourse.mybir as mybir
    import concourse.tile as tile
    import concourse.bacc as bacc
    from concourse.masks import make_identity
    from concourse import library_config

    D = meta["D"]
    bofs = meta["bofs"]
    calls = meta["calls"]
    group_cols = meta["group_cols"]
    COLS_MAX = meta["COLS_MAX"]
    gofs = meta["gofs"]
    seg_off = meta["seg_off"]
    gidx_cols_total = meta["gidx_cols_total"]
    TOTCOLS = meta["TOTCOLS"]

    f32 = mybir.dt.float32
    bf16 = mybir.dt.bfloat16
    ALU = mybir.AluOpType
    ACT = mybir.ActivationFunctionType

    nc = bacc.Bacc("TRN2", target_bir_lowering=False, debug=False,
                   num_devices=NCORE, num_swdge_queues=NQ)

    xT = nc.dram_tensor("xT", [IND, SPC], bf16, kind="ExternalInput")
    prew = nc.dram_tensor("prew", [IND, 66], f32, kind="ExternalInput")
    preb = nc.dram_tensor("preb", [P, 66], f32, kind="ExternalInput")
    gidx_t = nc.dram_tensor("gidx", [P, gidx_cols_total], mybir.dt.int16,
                            kind="ExternalInput")
    # rhs_l = [w_{l+1} | w_{l+1}@a_s | w_{l+1}@a_d]  (layers 1,2)
    rhs_t = [nc.dram_tensor(f"rhs{l}", [F, 66], f32, kind="ExternalInput")
             for l in (1, 2)]
    b_rep = [nc.dram_tensor(f"b{l}", [P, F], f32, kind="ExternalInput")
             for l in (1, 2, 3)]
    expand_t = nc.dram_tensor("expand", [GROUP, TOTCOLS], f32,
                              kind="ExternalInput")
    padfix = nc.dram_tensor("padfix", [1, 2], bf16, kind="ExternalInput")
    out_t = nc.dram_tensor("out", [SPC, F], f32, kind="ExternalOutput")

    # shard0 carries full 256B rows (phase-0 AllGathers straight into the
    # table); later shards carry only the 132B payload - their AllGather
    # lands in compact staging and a strided repack DMA expands into the
    # 256B-row gather table.
    shards = [nc.dram_tensor("shard0", [SPC, ESZ], bf16)] + [
        nc.dram_tensor(f"shard{l}", [SPC, 66], bf16) for l in (1, 2)]
    tables = [nc.dram_tensor(f"table{l}", [TROWS, ESZ], bf16,
                             addr_space="Shared") for l in range(3)]

    # collective chunks: list of (g0, g1) group ranges
    def mkranges(chunks):
        rr, g0 = [], 0
        for n_g in chunks:
            rr.append((g0, g0 + n_g))
            g0 += n_g
        assert g0 == NG
        return rr

    cc_ranges = mkranges(CC_CHUNKS)
    ph0_ranges = mkranges(PH0_CHUNKS)
    # contiguous compact staging tensors for chunked AllGather (layers 1,2)
    tstages = [None] + [
        [nc.dram_tensor(f"tstage{l}_{i}",
                        [NCORE * (g1 - g0) * GROUP * P, 66], bf16,
                        addr_space="Shared")
         for i, (g0, g1) in enumerate(cc_ranges)] for l in (1, 2)]

    with tile.TileContext(nc) as tc:
        with (
            tc.tile_pool(name="const", bufs=1) as cpool,
            tc.tile_pool(name="grid", bufs=3) as gpool,
            tc.tile_pool(name="small", bufs=2) as spool,
            tc.tile_pool(name="stage", bufs=1) as stpool,
            tc.tile_pool(name="psA", bufs=2, space="PSUM") as psA,
            tc.tile_pool(name="psB", bufs=2, space="PSUM") as psB,
            tc.tile_pool(name="psC", bufs=2, space="PSUM") as psC,
            tc.tile_pool(name="psD", bufs=2, space="PSUM") as psD,
        ):
            nc.gpsimd.load_library(library_config.mlp)

            ident_f = cpool.tile([P, P], f32)
            make_identity(nc, ident_f[:])
            ident_b = cpool.tile([P, P], bf16)
            nc.vector.tensor_copy(ident_b[:], ident_f[:])

            prew_sb = cpool.tile([P, 2 * 66], f32)
            nc.sync.dma_start(prew_sb[:, 0:66], prew[0:P, :])
            nc.sync.dma_start(prew_sb[:, 66:132], prew[P:2 * P, :])
            preb_sb = cpool.tile([P, 66], f32)
            nc.sync.dma_start(preb_sb[:], preb[:])
            rhs_sb = []
            for i in range(2):
                t = cpool.tile([F, 66], f32, tag=f"rhs{i}")
                nc.sync.dma_start(t[:], rhs_t[i][:])
                rhs_sb.append(t)
            b_sb = []
            for l in range(3):
                t = cpool.tile([P, F], f32, tag=f"bb{l}")
                nc.sync.dma_start(t[:], b_rep[l][:])
                b_sb.append(t)
            expand_sb = cpool.tile([GROUP, TOTCOLS], bf16)
            exp_f = cpool.tile([GROUP, TOTCOLS], f32, tag="expf")
            nc.sync.dma_start(exp_f[:], expand_t[:])
            nc.vector.tensor_copy(expand_sb[:], exp_f[:])

            hs_tiles = [cpool.tile([P, CPC], f32, tag=f"hs{i}", name=f"hs{i}")
                        for i in range(2)]
            hd_tiles = [cpool.tile([P, CPC], f32, tag=f"hd{i}", name=f"hd{i}")
                        for i in range(2)]
            # stage rows hold only the useful 66 bf16 elems; the shard's
            # trailing 62 elems are never written (gathered but unused)
            stages = [stpool.tile([P, CPC, 66], bf16, tag=f"st{i}",
                                  name=f"st{i}") for i in range(2)]

            def sl(g):
                return slice(g * GROUP, (g + 1) * GROUP)

            def publish(l, stage, stage_f32, cc_i, ranges):
                """DMA stage groups of chunk cc_i to shard l, AllGather into
                the contiguous staging tensor, repack into k-major table."""
                g0, g1 = ranges[cc_i]
                if g1 == NG:
                    nc.sync.dma_start(
                        stage[127:128, C_LAST, 64:66], padfix[:])
                r0, r1 = g0 * GROUP * P, g1 * GROUP * P
                if (r0, r1) == (0, SPC) and l == 0:
                    nc.scalar.dma_start(
                        shards[0][:].rearrange("(c p) w -> p c w",
                                               p=P)[:, :, 0:66],
                        stage[:, :, :])
                    nc.gpsimd.collective_compute(
                        "AllGather", ALU.bypass,
                        replica_groups=[list(range(NCORE))],
                        ins=[shards[0][:]], outs=[tables[0][:]])
                    return
                nc.scalar.dma_start(
                    shards[l][r0:r1].rearrange("(c p) w -> p c w", p=P),
                    stage[:, g0 * GROUP:g1 * GROUP, :])
                ts = tstages[l][cc_i]
                nc.gpsimd.collective_compute(
                    "AllGather", ALU.bypass,
                    replica_groups=[list(range(NCORE))],
                    ins=[shards[l][r0:r1]], outs=[ts[:]])
                nc.scalar.dma_start(
                    tables[l][:].rearrange("(k r) w -> k r w",
                                           k=NCORE)[:, r0:r1, 0:66],
                    ts[:].rearrange("(k r) w -> k r w", k=NCORE))


            prew_b = cpool.tile([P, 2 * 66], bf16)
            nc.vector.tensor_copy(prew_b[:], prew_sb[:])

            # ---------- phase 0: m1 = (x @ lin_w + lin_b) @ w1 (folded) ----
            stage0 = stages[0]
            stage0_f32 = stage0[:].bitcast(f32)
            cc0_done = 0
            for g in range(NG):
                u1ps = psB.tile([P, GROUP * 66], f32, space="PSUM", tag="msd")
                xts = []
                for h in range(2):
                    xt = spool.tile([P, GROUP * P], bf16, tag=f"xt{h}",
                                    name=f"xt{h}")
                    nc.sync.dma_start(
                        xt[:], xT[h * P:(h + 1) * P,
                                  g * GROUP * P:(g + 1) * GROUP * P])
                    xts.append(xt)
                for ci in range(GROUP):
                    for h in range(2):
                        nc.tensor.matmul(
                            out=u1ps[:, ci * 66:(ci + 1) * 66],
                            lhsT=xts[h][:, ci * P:(ci + 1) * P],
                            rhs=prew_b[:, h * 66:(h + 1) * 66],
                            start=(h == 0), stop=(h == 1))
                up = spool.tile([P, GROUP, 66], f32, tag="up")
                nc.vector.tensor_tensor(
                    out=up[:],
                    in0=u1ps[:].rearrange("p (c w) -> p c w", c=GROUP),
                    in1=preb_sb[:][:, None, :].to_broadcast([P, GROUP, 66]),
                    op=ALU.add)
                nc.vector.tensor_copy(stage0[:, sl(g), 0:F], up[:, :, 0:F])
                nc.vector.tensor_copy(
                    stage0_f32[:, sl(g), HS_F32COL:HS_F32COL + 1],
                    up[:, :, F:F + 1])
                nc.vector.tensor_copy(hs_tiles[0][:, sl(g)], up[:, :, F])
                nc.vector.tensor_copy(hd_tiles[0][:, sl(g)], up[:, :, F + 1])
                if g + 1 == ph0_ranges[cc0_done][1]:
                    publish(0, stage0, stage0_f32, cc0_done, ph0_ranges)
                    cc0_done += 1

            # ---------- layers ----------
            for l in range(3):
                table = tables[l]
                hs_cur = hs_tiles[l % 2]
                hd_cur = hd_tiles[l % 2]
                hs_nxt = hs_tiles[(l + 1) % 2]
                hd_nxt = hd_tiles[(l + 1) % 2]
                stage_cur = stages[l % 2]
                stage_nxt = stages[(l + 1) % 2]
                stage_nxt_f32 = stage_nxt[:].bitcast(f32)
                last = (l == 2)
                cc_done = 0
                for g in range(NG):
                    cols = int(group_cols[g])
                    grid = gpool.tile([P, COLS_MAX, ESZ], bf16, tag="grid")
                    gidx_sb = spool.tile([P, 8 * COLS_MAX], mybir.dt.int16,
                                         tag="gix")
                    gcall = [c for c in calls if c[0] == g]
                    gc_lo = gcall[0][4]
                    gc_hi = gcall[-1][4] + gcall[-1][3] * 8
                    nc.sync.dma_start(gidx_sb[:, 0:gc_hi - gc_lo],
                                      gidx_t[:, gc_lo:gc_hi])
                    for qi, (gg, b, w0, w, gc0) in enumerate(gcall):
                        nc.gpsimd.dma_gather(
                            grid[:, w0:w0 + w, :],
                            table[b * BROWS:(b + 1) * BROWS, :],
                            gidx_sb[:, gc0 - gc_lo:gc0 - gc_lo + w * 8],
                            w * P, w * P, ESZ,
                            queue_num=qi % NQ)
                    grid_f = grid[:].bitcast(f32)        # [P, COLS_MAX, 64]

                    # ---- d-expand: hdcols = hd_cur[:, g]^T-matmul expand ----
                    hdT = psD.tile([GROUP, P], f32, space="PSUM", tag="hdc")
                    nc.tensor.transpose(hdT[:], hd_cur[:, sl(g)], ident_f[:])
                    hdT_sb = spool.tile([GROUP, P], bf16, tag="hdTs")
                    nc.vector.tensor_copy(hdT_sb[:], hdT[:])
                    hdcols = psD.tile([P, COLS_MAX], f32, space="PSUM",
                                      tag="hdc")
                    nc.tensor.matmul(
                        out=hdcols[:, 0:cols], lhsT=hdT_sb[:],
                        rhs=expand_sb[:, gofs[g]:gofs[g] + cols],
                        start=True, stop=True)

                    # ---- e = lrelu(s_src + d_dst); ex = exp(e) ----
                    e_t = spool.tile([P, COLS_MAX], f32, tag="e")
                    nc.vector.tensor_tensor(
                        out=e_t[:, :cols], in0=grid_f[:, :cols, HS_F32COL],
                        in1=hdcols[:, :cols], op=ALU.add)
                    t02 = spool.tile([P, COLS_MAX], f32, tag="t02")
                    nc.vector.tensor_scalar_mul(t02[:, :cols], e_t[:, :cols],
                                                NEG)
                    nc.vector.tensor_tensor(out=e_t[:, :cols],
                                            in0=t02[:, :cols],
                                            in1=e_t[:, :cols], op=ALU.max)
                    ex_b = spool.tile([P, COLS_MAX], bf16, tag="exb")
                    nc.scalar.activation(ex_b[:, :cols], e_t[:, :cols],
                                         ACT.Exp)
                    nc.vector.tensor_tensor(
                        out=grid[:, :cols, 0:F],
                        in0=grid[:, :cols, 0:F],
                        in1=ex_b[:, :cols][:, :, None].to_broadcast(
                            [P, cols, F]),
                        op=ALU.mult)
                    nc.vector.tensor_copy(grid[:, :cols, F], ex_b[:, :cols])

                    # ---- aggregate [ex*m | ex] per chunk into PSUM ----
                    aggps = psA.tile([P, GROUP * RW], f32, space="PSUM",
                                     tag="aggps")
                    for ci in range(GROUP):
                        c = g * GROUP + ci
                        ncols = int(D[c].sum())
                        first = True
                        cnt = 0
                        for b in range(4):
                            w = int(D[c, b])
                            if w == 0:
                                continue
                            o = int(seg_off[c, b])
                            for j in range(w):
                                cnt += 1
                                nc.tensor.matmul(
                                    out=aggps[:, ci * RW:(ci + 1) * RW],
                                    lhsT=ident_b[:],
                                    rhs=grid[:, o + j, 0:RW],
                                    start=first, stop=(cnt == ncols))
                                first = False
                        if ncols == 0:
                            nc.vector.memset(
                                aggps[:, ci * RW:(ci + 1) * RW], 0.0)
                    agg3 = aggps[:].rearrange("p (c w) -> p c w", c=GROUP)

                    # ---- self-loop: exs = exp(lrelu(s_self + d_self)) ----
                    exs_e = spool.tile([P, GROUP], f32, tag="exse")
                    nc.vector.tensor_tensor(out=exs_e[:], in0=hs_cur[:, sl(g)],
                                            in1=hd_cur[:, sl(g)], op=ALU.add)
                    exs_t = spool.tile([P, GROUP], f32, tag="exst")
                    nc.vector.tensor_scalar_mul(exs_t[:], exs_e[:], NEG)
                    nc.vector.tensor_tensor(out=exs_e[:], in0=exs_t[:],
                                            in1=exs_e[:], op=ALU.max)
                    exs = spool.tile([P, GROUP], f32, tag="exs")
                    nc.scalar.activation(exs[:], exs_e[:], ACT.Exp)

                    mself = spool.tile([P, GROUP, F], f32, tag="mself")
                    nc.vector.tensor_copy(mself[:], stage_cur[:, sl(g), 0:F])
                    num = spool.tile([P, GROUP, F], f32, tag="num")
                    nc.vector.tensor_tensor(
                        out=num[:],
                        in0=mself[:],
                        in1=exs[:][:, :, None].to_broadcast([P, GROUP, F]),
                        op=ALU.mult)
                    nc.vector.tensor_tensor(out=num[:], in0=num[:],
                                            in1=agg3[:, :, 0:F], op=ALU.add)
                    den = spool.tile([P, GROUP], f32, tag="den")
                    nc.vector.tensor_tensor(out=den[:], in0=agg3[:, :, F],
                                            in1=exs[:], op=ALU.add)
                    nc.vector.tensor_scalar_add(den[:], den[:], 1e-16)
                    rec = spool.tile([P, GROUP], f32, tag="rec")
                    nc.vector.reciprocal(rec[:], den[:])
                    zb = spool.tile([P, GROUP, F], f32, tag="zb")
                    nc.vector.tensor_tensor(
                        out=zb[:], in0=num[:],
                        in1=rec[:][:, :, None].to_broadcast([P, GROUP, F]),
                        op=ALU.mult)
                    nc.vector.tensor_tensor(
                        out=zb[:], in0=zb[:],
                        in1=b_sb[l][:][:, None, :].to_broadcast([P, GROUP, F]),
                        op=ALU.add)

                    if not last:
                        h_sb = spool.tile([P, GROUP, F], f32, tag="h")
                        nc.vector.tensor_scalar_max(h_sb[:], zb[:], 0.0)
                        msd = psB.tile([P, GROUP * 66], f32, space="PSUM",
                                       tag="msd")
                        for ci in range(GROUP):
                            ht = psC.tile([F, P], f32, space="PSUM", tag="ht")
                            nc.tensor.transpose(ht[:], h_sb[:, ci, :],
                                                ident_f[:])
                            ht_sb = spool.tile([F, P], f32, tag="hts")
                            nc.vector.tensor_copy(ht_sb[:], ht[:])
                            nc.tensor.matmul(
                                out=msd[:, ci * 66:(ci + 1) * 66],
                                lhsT=ht_sb[:], rhs=rhs_sb[l][:],
                                start=True, stop=True)
                        msd3 = msd[:].rearrange("p (c w) -> p c w", c=GROUP)
                        nc.vector.tensor_copy(stage_nxt[:, sl(g), 0:F],
                                              msd3[:, :, 0:F])
                        nc.vector.tensor_copy(
                            stage_nxt_f32[:, sl(g), HS_F32COL:HS_F32COL + 1],
                            msd3[:, :, F:F + 1])
                        nc.vector.tensor_copy(hs_nxt[:, sl(g)], msd3[:, :, F])
                        nc.vector.tensor_copy(hd_nxt[:, sl(g)],
                                              msd3[:, :, F + 1])
                    else:
                        sq = spool.tile([P, GROUP, F], f32, tag="h")
                        nc.vector.tensor_tensor(out=sq[:], in0=zb[:],
                                                in1=zb[:], op=ALU.mult)
                        n2 = spool.tile([P, GROUP], f32, tag="den")
                        nc.vector.reduce_sum(n2[:], sq[:],
                                             axis=mybir.AxisListType.X)
                        nrm = spool.tile([P, GROUP], f32, tag="rec")
                        nc.scalar.activation(nrm[:], n2[:], ACT.Sqrt)
                        nc.vector.tensor_scalar_max(nrm[:], nrm[:], 1e-12)
                        rinv = spool.tile([P, GROUP], f32, tag="nrmi")
                        nc.vector.reciprocal(rinv[:], nrm[:])
                        ob = spool.tile([P, GROUP, F], f32, tag="ob")
                        nc.vector.tensor_tensor(
                            out=ob[:], in0=zb[:],
                            in1=rinv[:][:, :, None].to_broadcast(
                                [P, GROUP, F]),
                            op=ALU.mult)
                        nc.sync.dma_start(
                            out_t[g * GROUP * P:(g + 1) * GROUP * P, :]
                            .rearrange("(c p) w -> p c w", p=P),
                            ob[:])

                    while (not last and cc_done < len(cc_ranges)
                           and g >= min(cc_ranges[cc_done][1] - 1 + CC_DELAY,
                                        NG - 1)):
                        publish(l + 1, stage_nxt, stage_nxt_f32, cc_done,
                                cc_ranges)
                        cc_done += 1

    nc.compile()
    return nc


# ======================= driver =======================

def kernel(x, src, dst, lin_w, lin_b, w1, a1s, a1d, b1, w2, a2s, a2d, b2,
           w3, a3s, a3d, b3):
    if "built" not in _cache:
        meta = _preprocess(src, dst)
        nc = _build_nc(meta)
        _cache["built"] = (meta, nc)
    meta, nc = _cache["built"]

    x = np.asarray(x, np.float32)
    lin_w = np.asarray(lin_w, np.float32)
    lin_b = np.asarray(lin_b, np.float32)
    ws = {1: np.asarray(w1, np.float32), 2: np.asarray(w2, np.float32),
          3: np.asarray(w3, np.float32)}
    avs = {1: (np.asarray(a1s, np.float32), np.asarray(a1d, np.float32)),
           2: (np.asarray(a2s, np.float32), np.asarray(a2d, np.float32)),
           3: (np.asarray(a3s, np.float32), np.asarray(a3d, np.float32))}
    bs = {1: np.asarray(b1, np.float32), 2: np.asarray(b2, np.float32),
          3: np.asarray(b3, np.float32)}

    node_at = meta["node_at"]
    newid = meta["newid"]
    gidx = meta["gidx"]

    # layer-1 message fold: m1 = (x@lin_w + lin_b) @ w1; s/d scores
    W1 = lin_w @ ws[1]                         # [256, 64]
    prew = np.concatenate(
        [W1, (W1 @ avs[1][0])[:, None], (W1 @ avs[1][1])[:, None]],
        axis=1).astype(np.float32)
    pb = lin_b @ ws[1]
    preb_row = np.concatenate([pb, [pb @ avs[1][0]], [pb @ avs[1][1]]])
    preb = np.tile(preb_row[None, :].astype(np.float32), (P, 1))
    ins_shared = {"prew": prew, "preb": preb, "expand": meta["expand"],
                  "padfix": np.frombuffer(np.float32(-1e5).tobytes(),
                                          dtype=ml_dtypes.bfloat16
                                          ).reshape(1, 2).copy()}
    for i, l in enumerate((2, 3)):
        wl = ws[l]
        ins_shared[f"rhs{i + 1}"] = np.concatenate(
            [wl, (wl @ avs[l][0])[:, None], (wl @ avs[l][1])[:, None]],
            axis=1).astype(np.float32)
    for l in (1, 2, 3):
        ins_shared[f"b{l}"] = np.tile(bs[l][None, :], (P, 1)).astype(np.float32)

    in_maps = []
    for k in range(NCORE):
        nodes = node_at[k].reshape(-1)
        xk = np.zeros((SPC, IND), np.float32)
        valid = nodes >= 0
        xk[valid] = x[nodes[valid]]
        m = dict(ins_shared)
        m["xT"] = np.ascontiguousarray(xk.T).astype(ml_dtypes.bfloat16)
        m["gidx"] = np.ascontiguousarray(gidx[k])
        in_maps.append(m)

    from concourse.bass_utils import run_bass_kernel_spmd
    import os
    trace = bool(os.environ.get("BASS_TRACE"))
    res = run_bass_kernel_spmd(nc, in_maps, list(range(NCORE)), trace=trace)
    if trace:
        globals()["LAST_EXEC_NS"] = res.exec_time_ns
        globals()["LAST_TRACE"] = (res.instructions_and_trace or (None, None))[1]
        globals()["LAST_PROFILE_JSON"] = res.profile_json

    out_new = np.concatenate([res.results[k]["out"] for k in range(NCORE)],
                             axis=0)
    return out_new[newid].astype(np.float32)


# revision 34
# speedup vs baseline: 1.2085x; 1.2085x over previous
"""3-layer GAT (nn_GAT_64003602645176) on 8 TRN2 NeuronCores via Bass.

Sharding: nodes across 8 cores, edges partitioned by dst.  Per layer every
core holds a full AllGather'd table of rows [m_l (64 bf16) | s_l (1 f32) |
pad] (256B rows) where m_l = h_l @ w_{l+1} is the NEXT layer's message and
s_l = m_l @ a_s its source score; the per-layer linear is folded into the
table (GAT aggregation is linear in w).  Edge gathers via gpsimd.dma_gather
(int16 idx, 4 row-banks of 25088 rows, 2 SWDGE queues).  Chunk packing via
global shared-shape set-cover so all 8 cores share one column-width profile
(low padding + balanced load).  Aggregation via identity-lhsT matmuls
accumulating [ex*m | ex] into PSUM per 128-node chunk.  Self-loops are not
gathered: their contribution is added from the local stage tile.  Softmax
without max subtraction (scores are small); dummy slots gather a row whose
s is -1e5 so their weight is 0.  Per-column dst scores d are expanded from
per-chunk values with a static 0/1 matmul.  Tables are chunk-major so the
AllGather can be split into strided-output chunks overlapping compute.
"""
import sys
import numpy as np

sys.path.insert(0, "/opt/trn_rl_repo")
import ml_dtypes  # noqa: E402,F401

N, E = 100000, 1280000
IND, F = 256, 64
NEG = 0.2
NCORE = 8
NPC = 12500
P = 128
CPC = 98
SPC = CPC * P               # 12544 table rows per core
BROWS = 2 * SPC             # 25088 rows per bank
TROWS = NCORE * SPC
ESZ = 128                   # bf16 elems per row (256B)
HS_F32COL = 32
GROUP = 7
NG = CPC // GROUP
RW = 65
DMAX = 20
GCALL_COLS = 8              # dest cols per dma_gather call (1024 idxs)
NQ = 4                      # SWDGE queues
CC_CHUNKS = [5, 4, 3, 2]    # groups per AllGather chunk; each chunk gathers
                            # into a contiguous staging tensor, then a local
                            # strided DMA repacks it into the k-major table
CC_DELAY = 1                # publish a chunk this many groups after its data
                            # is ready so the in-order gpsimd queue never
                            # stalls waiting for the chunk's compute
PH0_CHUNKS = [14]           # phase 0 has no gather stream to hide behind:
                            # one full-table AllGather (no repack) is cheaper
C_LAST = CPC - 1
DUMMY_LOCAL = C_LAST * P + 127   # chunk-major local row within even core

_cache = {}


# ======================= host preprocessing =======================

def _preprocess(src, dst):
    s = np.asarray(src, np.int64) % N
    d = np.asarray(dst, np.int64) % N
    deg = np.bincount(d, minlength=N)

    # ---- node -> core: LPT on in-degree ----
    import heapq
    order = np.argsort(-deg, kind="stable")
    heap = [(0, 0, k) for k in range(NCORE)]
    heapq.heapify(heap)
    core_of = np.empty(N, np.int32)
    for n in order:
        load, cnt, k = heapq.heappop(heap)
        core_of[n] = k
        if cnt + 1 < NPC:
            heapq.heappush(heap, (load + int(deg[n]), cnt + 1, k))
    bank_of_node = core_of // 2

    eb = bank_of_node[s]
    db4 = np.zeros((N, 4), np.int64)
    for b in range(4):
        db4[:, b] = np.bincount(d[eb == b], minlength=N)

    # ---- global shared-shape set-cover packing ----
    profs = [db4[core_of == k] for k in range(NCORE)]
    nodes_of = [np.where(core_of == k)[0] for k in range(NCORE)]
    clipped = [np.minimum(p, DMAX) for p in profs]
    unassigned = [np.ones(len(p), bool) for p in profs]
    asg = [np.full(len(p), -1, np.int32) for p in profs]
    Wenv = np.zeros((CPC, 4), np.int64)
    # reserved dummy position: (box C_LAST, partition 127) kept empty;
    # box C_LAST capacity 127 on every core.
    caps = np.full(CPC, P, np.int64)

    rng_cost = (np.arange(DMAX + 1)[:, None, None, None]
                + np.arange(DMAX + 1)[None, :, None, None]
                + np.arange(DMAX + 1)[None, None, :, None]
                + np.arange(DMAX + 1)[None, None, None, :]).astype(np.float64)

    def count_tensors():
        Cs = []
        for k in range(NCORE):
            C = np.zeros((DMAX + 1,) * 4, np.int32)
            idx = clipped[k][unassigned[k]]
            np.add.at(C, (idx[:, 0], idx[:, 1], idx[:, 2], idx[:, 3]), 1)
            Cs.append(C.cumsum(0).cumsum(1).cumsum(2).cumsum(3))
        return Cs

    nbox = 0
    while nbox < CPC:
        rem_boxes = CPC - nbox
        Cs = count_tensors()
        fit_min = np.minimum.reduce(Cs)
        rem_nodes_max = max(int(u.sum()) for u in unassigned)
        need_fill = rem_nodes_max - (rem_boxes - 1) * P
        fit = np.minimum(fit_min, P)
        ratio = fit / np.maximum(rng_cost, 1)
        ratio[fit < min(P, max(need_fill, 1))] = -1
        if ratio.max() <= 0:
            ratio = np.minimum(fit_min, P) / np.maximum(rng_cost, 1)
        shape = np.array(np.unravel_index(np.argmax(ratio), ratio.shape),
                         np.int64)
        for k in range(NCORE):
            cand = np.where(unassigned[k]
                            & (clipped[k] <= shape[None, :]).all(1))[0]
            cand = cand[np.argsort(-profs[k][cand].sum(1),
                                   kind="stable")][:P]
            asg[k][cand] = nbox
            unassigned[k][cand] = False
            if len(cand):
                Wenv[nbox] = np.maximum(Wenv[nbox], profs[k][cand].max(0))
        nbox += 1

    # leftovers -> boxes with space, min growth (box C_LAST holds <=127)
    for k in range(NCORE):
        left = np.where(unassigned[k])[0]
        if not len(left):
            continue
        left = left[np.argsort(-profs[k][left].sum(1))]
        space = np.bincount(asg[k][asg[k] >= 0], minlength=CPC)
        for n_i in left:
            pv = profs[k][n_i]
            grow = np.maximum(pv[None, :] - Wenv, 0).sum(1).astype(np.float64)
            limit = np.where(np.arange(CPC) == C_LAST, P - 1, P)
            grow[space >= limit] = np.inf
            c = int(np.argmin(grow))
            asg[k][n_i] = c
            space[c] += 1
            Wenv[c] = np.maximum(Wenv[c], pv)
    # enforce reserved dummy slot: if any core filled box C_LAST to 128,
    # move one node out (should not happen due to limit above)
    for k in range(NCORE):
        mem = np.where(asg[k] == C_LAST)[0]
        assert len(mem) <= P - 1

    # ---- box -> group assignment (LPT balance on total width, 7/group) ----
    totw = Wenv.sum(1)
    order_b = np.argsort(-totw, kind="stable")
    gload = np.zeros(NG, np.int64)
    gcnt = np.zeros(NG, np.int64)
    slot_of_box = np.empty(CPC, np.int64)
    gslots = [[] for _ in range(NG)]
    for b_i in order_b:
        if b_i == C_LAST:
            continue
        g = min((g for g in range(NG) if gcnt[g] < GROUP
                 - (1 if g == NG - 1 else 0)),
                key=lambda g: (gload[g], gcnt[g]))
        gslots[g].append(b_i)
        gload[g] += totw[b_i]
        gcnt[g] += 1
    gslots[NG - 1].append(C_LAST)   # dummy box last
    # relabel groups by descending load so the tail groups are light (their
    # compute finishes fast, shrinking the last AllGather chunk's latency);
    # keep the dummy-box group last regardless.
    order_g = sorted(range(NG - 1), key=lambda g: -gload[g]) + [NG - 1]
    gslots = [gslots[g] for g in order_g]
    for g in range(NG):
        assert len(gslots[g]) == GROUP
        for j, b_i in enumerate(gslots[g]):
            slot_of_box[b_i] = g * GROUP + j
    # remap: slot c (0..97); D widths per slot
    D = np.zeros((CPC, 4), np.int64)
    D[slot_of_box] = Wenv
    new_c_last = slot_of_box[C_LAST]
    assert new_c_last == CPC - 1

    # ---- node_at [core, slot, p] ----
    node_at = np.full((NCORE, CPC, P), -1, np.int64)
    for k in range(NCORE):
        slots = slot_of_box[asg[k]]
        for c in range(CPC):
            mem = nodes_of[k][slots == c]
            mem = mem[np.argsort(-deg[mem], kind="stable")]
            node_at[k, c, :len(mem)] = mem
    assert (node_at[:, C_LAST, 127] == -1).all()

    # table row (chunk-major within core): k*SPC + c*P + p
    tbl_row = np.empty(N, np.int64)
    k_idx, c_idx, p_idx = np.nonzero(node_at >= 0)
    orig = node_at[k_idx, c_idx, p_idx]
    tbl_row[orig] = k_idx * SPC + c_idx * P + p_idx
    newid = np.empty(N, np.int64)
    newid[orig] = k_idx * SPC + c_idx * P + p_idx

    # ---- per-group column layout: (b, c, j), bank-major ----
    group_cols = np.zeros(NG, np.int64)
    bofs = np.zeros((NG, 5), np.int64)
    seg_off = np.zeros((CPC, 4), np.int64)    # column offset of (c,b) seg
    for g in range(NG):
        o = 0
        for b in range(4):
            bofs[g, b] = o
            for c in range(g * GROUP, (g + 1) * GROUP):
                seg_off[c, b] = o
                o += int(D[c, b])
        bofs[g, 4] = o
        group_cols[g] = o
    COLS_MAX = int(group_cols.max())
    gofs = np.concatenate([[0], np.cumsum(group_cols)])
    TOTCOLS = int(group_cols.sum())

    # ---- edge sort & idx grid ----
    e_order = np.lexsort((tbl_row[s], eb, d))
    ds, ss, ebs = d[e_order], s[e_order], eb[e_order]
    keys = ds * 4 + ebs
    starts = np.searchsorted(
        keys, np.arange(N, dtype=np.int64)[:, None] * 4 + np.arange(4)[None, :],
        side="left")

    idx_grid = np.full((NCORE, TOTCOLS, P), DUMMY_LOCAL, np.int64)
    for k in range(NCORE):
        for c in range(CPC):
            g = c // GROUP
            nodes = node_at[k, c]
            nv = np.where(nodes >= 0)[0]
            nn = nodes[nv]
            for b in range(4):
                J = int(D[c, b])
                if J == 0:
                    continue
                cnt = db4[nn, b]
                jj = np.arange(J)[:, None]
                has = jj < cnt[None, :]
                st = np.minimum(starts[nn, b][None, :] + jj, len(ss) - 1)
                rows = tbl_row[ss[st]] - b * BROWS
                rows = np.where(has, rows, DUMMY_LOCAL)
                g0 = gofs[g] + seg_off[c, b]
                idx_grid[k, g0:g0 + J][:, nv] = rows
    assert idx_grid.min() >= 0 and idx_grid.max() < BROWS
    idx_grid = idx_grid.astype(np.int16)

    # gather calls: per (g, b) segment split into <=GCALL_COLS columns
    calls = []
    gidx_cols_total = 0
    for g in range(NG):
        for b in range(4):
            c0, c1 = int(bofs[g, b]), int(bofs[g, b + 1])
            w0 = c0
            while w0 < c1:
                w = min(GCALL_COLS, c1 - w0)
                calls.append((g, b, w0, w, gidx_cols_total))
                gidx_cols_total += w * 8
                w0 += w
    gidx = np.zeros((NCORE, P, gidx_cols_total), np.int16)
    for k in range(NCORE):
        for (g, b, w0, w, gc0) in calls:
            flat = idx_grid[k, gofs[g] + w0: gofs[g] + w0 + w].reshape(-1)
            wrapped = flat.reshape(-1, 16).T
            gidx[k, :, gc0:gc0 + w * 8] = np.tile(wrapped, (8, 1))

    # expand matrices: [7, TOTCOLS], 1 at (c_local, col) if col in seg of c
    expand = np.zeros((GROUP, TOTCOLS), np.float32)
    for g in range(NG):
        for b in range(4):
            for ci in range(GROUP):
                c = g * GROUP + ci
                o = gofs[g] + seg_off[c, b]
                expand[ci, o:o + int(D[c, b])] = 1.0

    return dict(node_at=node_at, D=D, newid=newid, group_cols=group_cols,
                COLS_MAX=COLS_MAX, bofs=bofs, gofs=gofs, seg_off=seg_off,
                calls=calls, gidx=gidx, gidx_cols_total=gidx_cols_total,
                expand=expand, TOTCOLS=TOTCOLS)


# ======================= device program =======================

def _build_nc(meta):
    import concourse.bass as bass  # noqa: F401
    import concourse.mybir as mybir
    import concourse.tile as tile
    import concourse.bacc as bacc
    from concourse.masks import make_identity
    from concourse import library_config

    D = meta["D"]
    bofs = meta["bofs"]
    calls = meta["calls"]
    group_cols = meta["group_cols"]
    COLS_MAX = meta["COLS_MAX"]
    gofs = meta["gofs"]
    seg_off = meta["seg_off"]
    gidx_cols_total = meta["gidx_cols_total"]
    TOTCOLS = meta["TOTCOLS"]

    f32 = mybir.dt.float32
    bf16 = mybir.dt.bfloat16
    ALU = mybir.AluOpType
    ACT = mybir.ActivationFunctionType

    nc = bacc.Bacc("TRN2", target_bir_lowering=False, debug=False,
                   num_devices=NCORE, num_swdge_queues=NQ)

    xT = nc.dram_tensor("xT", [IND, SPC], bf16, kind="ExternalInput")
    prew = nc.dram_tensor("prew", [IND, 66], f32, kind="ExternalInput")
    preb = nc.dram_tensor("preb", [P, 66], f32, kind="ExternalInput")
    gidx_t = nc.dram_tensor("gidx", [P, gidx_cols_total], mybir.dt.int16,
                            kind="ExternalInput")
    # rhs_l = [w_{l+1} | w_{l+1}@a_s | w_{l+1}@a_d]  (layers 1,2)
    rhs_t = [nc.dram_tensor(f"rhs{l}", [F, 66], f32, kind="ExternalInput")
             for l in (1, 2)]
    b_rep = [nc.dram_tensor(f"b{l}", [P, F], f32, kind="ExternalInput")
             for l in (1, 2, 3)]
    expand_t = nc.dram_tensor("expand", [GROUP, TOTCOLS], f32,
                              kind="ExternalInput")
    padfix = nc.dram_tensor("padfix", [1, 2], bf16, kind="ExternalInput")
    out_t = nc.dram_tensor("out", [SPC, F], f32, kind="ExternalOutput")

    shards = [nc.dram_tensor(f"shard{l}", [SPC, ESZ], bf16)
              for l in range(3)]
    tables = [nc.dram_tensor(f"table{l}", [TROWS, ESZ], bf16,
                             addr_space="Shared") for l in range(3)]

    # collective chunks: list of (g0, g1) group ranges
    def mkranges(chunks):
        rr, g0 = [], 0
        for n_g in chunks:
            rr.append((g0, g0 + n_g))
            g0 += n_g
        assert g0 == NG
        return rr

    cc_ranges = mkranges(CC_CHUNKS)
    ph0_ranges = mkranges(PH0_CHUNKS)
    # contiguous staging tensors for chunked AllGather (layers 1,2 only)
    tstages = [None] + [
        [nc.dram_tensor(f"tstage{l}_{i}",
                        [NCORE * (g1 - g0) * GROUP * P, ESZ], bf16,
                        addr_space="Shared")
         for i, (g0, g1) in enumerate(cc_ranges)] for l in (1, 2)]

    with tile.TileContext(nc) as tc:
        with (
            tc.tile_pool(name="const", bufs=1) as cpool,
            tc.tile_pool(name="grid", bufs=3) as gpool,
            tc.tile_pool(name="small", bufs=2) as spool,
            tc.tile_pool(name="stage", bufs=1) as stpool,
            tc.tile_pool(name="psA", bufs=2, space="PSUM") as psA,
            tc.tile_pool(name="psB", bufs=2, space="PSUM") as psB,
            tc.tile_pool(name="psC", bufs=2, space="PSUM") as psC,
            tc.tile_pool(name="psD", bufs=2, space="PSUM") as psD,
        ):
            nc.gpsimd.load_library(library_config.mlp)

            ident_f = cpool.tile([P, P], f32)
            make_identity(nc, ident_f[:])
            ident_b = cpool.tile([P, P], bf16)
            nc.vector.tensor_copy(ident_b[:], ident_f[:])

            prew_sb = cpool.tile([P, 2 * 66], f32)
            nc.sync.dma_start(prew_sb[:, 0:66], prew[0:P, :])
            nc.sync.dma_start(prew_sb[:, 66:132], prew[P:2 * P, :])
            preb_sb = cpool.tile([P, 66], f32)
            nc.sync.dma_start(preb_sb[:], preb[:])
            rhs_sb = []
            for i in range(2):
                t = cpool.tile([F, 66], f32, tag=f"rhs{i}")
                nc.sync.dma_start(t[:], rhs_t[i][:])
                rhs_sb.append(t)
            b_sb = []
            for l in range(3):
                t = cpool.tile([P, F], f32, tag=f"bb{l}")
                nc.sync.dma_start(t[:], b_rep[l][:])
                b_sb.append(t)
            expand_sb = cpool.tile([GROUP, TOTCOLS], bf16)
            exp_f = cpool.tile([GROUP, TOTCOLS], f32, tag="expf")
            nc.sync.dma_start(exp_f[:], expand_t[:])
            nc.vector.tensor_copy(expand_sb[:], exp_f[:])

            hs_tiles = [cpool.tile([P, CPC], f32, tag=f"hs{i}", name=f"hs{i}")
                        for i in range(2)]
            hd_tiles = [cpool.tile([P, CPC], f32, tag=f"hd{i}", name=f"hd{i}")
                        for i in range(2)]
            # stage rows hold only the useful 66 bf16 elems; the shard's
            # trailing 62 elems are never written (gathered but unused)
            stages = [stpool.tile([P, CPC, 66], bf16, tag=f"st{i}",
                                  name=f"st{i}") for i in range(2)]

            def sl(g):
                return slice(g * GROUP, (g + 1) * GROUP)

            def publish(l, stage, stage_f32, cc_i, ranges):
                """DMA stage groups of chunk cc_i to shard l, AllGather into
                the contiguous staging tensor, repack into k-major table."""
                g0, g1 = ranges[cc_i]
                if g1 == NG:
                    nc.sync.dma_start(
                        stage[127:128, C_LAST, 64:66], padfix[:])
                r0, r1 = g0 * GROUP * P, g1 * GROUP * P
                nc.scalar.dma_start(
                    shards[l][r0:r1].rearrange("(c p) w -> p c w",
                                               p=P)[:, :, 0:66],
                    stage[:, g0 * GROUP:g1 * GROUP, :])
                if (r0, r1) == (0, SPC):
                    nc.gpsimd.collective_compute(
                        "AllGather", ALU.bypass,
                        replica_groups=[list(range(NCORE))],
                        ins=[shards[l][r0:r1]], outs=[tables[l][:]])
                    return
                ts = tstages[l][cc_i]
                nc.gpsimd.collective_compute(
                    "AllGather", ALU.bypass,
                    replica_groups=[list(range(NCORE))],
                    ins=[shards[l][r0:r1]], outs=[ts[:]])
                nc.scalar.dma_start(
                    tables[l][:].rearrange("(k r) w -> k r w",
                                           k=NCORE)[:, r0:r1, :],
                    ts[:].rearrange("(k r) w -> k r w", k=NCORE))


            prew_b = cpool.tile([P, 2 * 66], bf16)
            nc.vector.tensor_copy(prew_b[:], prew_sb[:])

            # ---------- phase 0: m1 = (x @ lin_w + lin_b) @ w1 (folded) ----
            stage0 = stages[0]
            stage0_f32 = stage0[:].bitcast(f32)
            cc0_done = 0
            for g in range(NG):
                u1ps = psB.tile([P, GROUP * 66], f32, space="PSUM", tag="msd")
                xts = []
                for h in range(2):
                    xt = spool.tile([P, GROUP * P], bf16, tag=f"xt{h}",
                                    name=f"xt{h}")
                    nc.sync.dma_start(
                        xt[:], xT[h * P:(h + 1) * P,
                                  g * GROUP * P:(g + 1) * GROUP * P])
                    xts.append(xt)
                for ci in range(GROUP):
                    for h in range(2):
                        nc.tensor.matmul(
                            out=u1ps[:, ci * 66:(ci + 1) * 66],
                            lhsT=xts[h][:, ci * P:(ci + 1) * P],
                            rhs=prew_b[:, h * 66:(h + 1) * 66],
                            start=(h == 0), stop=(h == 1))
                up = spool.tile([P, GROUP, 66], f32, tag="up")
                nc.vector.tensor_tensor(
                    out=up[:],
                    in0=u1ps[:].rearrange("p (c w) -> p c w", c=GROUP),
                    in1=preb_sb[:][:, None, :].to_broadcast([P, GROUP, 66]),
                    op=ALU.add)
                nc.vector.tensor_copy(stage0[:, sl(g), 0:F], up[:, :, 0:F])
                nc.vector.tensor_copy(
                    stage0_f32[:, sl(g), HS_F32COL:HS_F32COL + 1],
                    up[:, :, F:F + 1])
                nc.vector.tensor_copy(hs_tiles[0][:, sl(g)], up[:, :, F])
                nc.vector.tensor_copy(hd_tiles[0][:, sl(g)], up[:, :, F + 1])
                if g + 1 == ph0_ranges[cc0_done][1]:
                    publish(0, stage0, stage0_f32, cc0_done, ph0_ranges)
                    cc0_done += 1

            # ---------- layers ----------
            for l in range(3):
                table = tables[l]
                hs_cur = hs_tiles[l % 2]
                hd_cur = hd_tiles[l % 2]
                hs_nxt = hs_tiles[(l + 1) % 2]
                hd_nxt = hd_tiles[(l + 1) % 2]
                stage_cur = stages[l % 2]
                stage_nxt = stages[(l + 1) % 2]
                stage_nxt_f32 = stage_nxt[:].bitcast(f32)
                last = (l == 2)
                cc_done = 0
                for g in range(NG):
                    cols = int(group_cols[g])
                    grid = gpool.tile([P, COLS_MAX, ESZ], bf16, tag="grid")
                    gidx_sb = spool.tile([P, 8 * COLS_MAX], mybir.dt.int16,
                                         tag="gix")
                    gcall = [c for c in calls if c[0] == g]
                    gc_lo = gcall[0][4]
                    gc_hi = gcall[-1][4] + gcall[-1][3] * 8
                    nc.sync.dma_start(gidx_sb[:, 0:gc_hi - gc_lo],
                                      gidx_t[:, gc_lo:gc_hi])
                    for qi, (gg, b, w0, w, gc0) in enumerate(gcall):
                        nc.gpsimd.dma_gather(
                            grid[:, w0:w0 + w, :],
                            table[b * BROWS:(b + 1) * BROWS, :],
                            gidx_sb[:, gc0 - gc_lo:gc0 - gc_lo + w * 8],
                            w * P, w * P, ESZ,
                            queue_num=qi % NQ)
                    grid_f = grid[:].bitcast(f32)        # [P, COLS_MAX, 64]

                    # ---- d-expand: hdcols = hd_cur[:, g]^T-matmul expand ----
                    hdT = psD.tile([GROUP, P], f32, space="PSUM", tag="hdc")
                    nc.tensor.transpose(hdT[:], hd_cur[:, sl(g)], ident_f[:])
                    hdT_sb = spool.tile([GROUP, P], bf16, tag="hdTs")
                    nc.vector.tensor_copy(hdT_sb[:], hdT[:])
                    hdcols = psD.tile([P, COLS_MAX], f32, space="PSUM",
                                      tag="hdc")
                    nc.tensor.matmul(
                        out=hdcols[:, 0:cols], lhsT=hdT_sb[:],
                        rhs=expand_sb[:, gofs[g]:gofs[g] + cols],
                        start=True, stop=True)

                    # ---- e = lrelu(s_src + d_dst); ex = exp(e) ----
                    e_t = spool.tile([P, COLS_MAX], f32, tag="e")
                    nc.vector.tensor_tensor(
                        out=e_t[:, :cols], in0=grid_f[:, :cols, HS_F32COL],
                        in1=hdcols[:, :cols], op=ALU.add)
                    t02 = spool.tile([P, COLS_MAX], f32, tag="t02")
                    nc.vector.tensor_scalar_mul(t02[:, :cols], e_t[:, :cols],
                                                NEG)
                    nc.vector.tensor_tensor(out=e_t[:, :cols],
                                            in0=t02[:, :cols],
                                            in1=e_t[:, :cols], op=ALU.max)
                    ex_b = spool.tile([P, COLS_MAX], bf16, tag="exb")
                    nc.scalar.activation(ex_b[:, :cols], e_t[:, :cols],
                                         ACT.Exp)
                    nc.vector.tensor_tensor(
                        out=grid[:, :cols, 0:F],
                        in0=grid[:, :cols, 0:F],
                        in1=ex_b[:, :cols][:, :, None].to_broadcast(
                            [P, cols, F]),
                        op=ALU.mult)
                    nc.vector.tensor_copy(grid[:, :cols, F], ex_b[:, :cols])

                    # ---- aggregate [ex*m | ex] per chunk into PSUM ----
                    aggps = psA.tile([P, GROUP * RW], f32, space="PSUM",
                                     tag="aggps")
                    for ci in range(GROUP):
                        c = g * GROUP + ci
                        ncols = int(D[c].sum())
                        first = True
                        cnt = 0
                        for b in range(4):
                            w = int(D[c, b])
                            if w == 0:
                                continue
                            o = int(seg_off[c, b])
                            for j in range(w):
                                cnt += 1
                                nc.tensor.matmul(
                                    out=aggps[:, ci * RW:(ci + 1) * RW],
                                    lhsT=ident_b[:],
                                    rhs=grid[:, o + j, 0:RW],
                                    start=first, stop=(cnt == ncols))
                                first = False
                        if ncols == 0:
                            nc.vector.memset(
                                aggps[:, ci * RW:(ci + 1) * RW], 0.0)
                    agg3 = aggps[:].rearrange("p (c w) -> p c w", c=GROUP)

                    # ---- self-loop: exs = exp(lrelu(s_self + d_self)) ----
                    exs_e = spool.tile([P, GROUP], f32, tag="exse")
                    nc.vector.tensor_tensor(out=exs_e[:], in0=hs_cur[:, sl(g)],
                                            in1=hd_cur[:, sl(g)], op=ALU.add)
                    exs_t = spool.tile([P, GROUP], f32, tag="exst")
                    nc.vector.tensor_scalar_mul(exs_t[:], exs_e[:], NEG)
                    nc.vector.tensor_tensor(out=exs_e[:], in0=exs_t[:],
                                            in1=exs_e[:], op=ALU.max)
                    exs = spool.tile([P, GROUP], f32, tag="exs")
                    nc.scalar.activation(exs[:], exs_e[:], ACT.Exp)

                    mself = spool.tile([P, GROUP, F], f32, tag="mself")
                    nc.vector.tensor_copy(mself[:], stage_cur[:, sl(g), 0:F])
                    num = spool.tile([P, GROUP, F], f32, tag="num")
                    nc.vector.tensor_tensor(
                        out=num[:],
                        in0=mself[:],
                        in1=exs[:][:, :, None].to_broadcast([P, GROUP, F]),
                        op=ALU.mult)
                    nc.vector.tensor_tensor(out=num[:], in0=num[:],
                                            in1=agg3[:, :, 0:F], op=ALU.add)
                    den = spool.tile([P, GROUP], f32, tag="den")
                    nc.vector.tensor_tensor(out=den[:], in0=agg3[:, :, F],
                                            in1=exs[:], op=ALU.add)
                    nc.vector.tensor_scalar_add(den[:], den[:], 1e-16)
                    rec = spool.tile([P, GROUP], f32, tag="rec")
                    nc.vector.reciprocal(rec[:], den[:])
                    zb = spool.tile([P, GROUP, F], f32, tag="zb")
                    nc.vector.tensor_tensor(
                        out=zb[:], in0=num[:],
                        in1=rec[:][:, :, None].to_broadcast([P, GROUP, F]),
                        op=ALU.mult)
                    nc.vector.tensor_tensor(
                        out=zb[:], in0=zb[:],
                        in1=b_sb[l][:][:, None, :].to_broadcast([P, GROUP, F]),
                        op=ALU.add)

                    if not last:
                        h_sb = spool.tile([P, GROUP, F], f32, tag="h")
                        nc.vector.tensor_scalar_max(h_sb[:], zb[:], 0.0)
                        msd = psB.tile([P, GROUP * 66], f32, space="PSUM",
                                       tag="msd")
                        for ci in range(GROUP):
                            ht = psC.tile([F, P], f32, space="PSUM", tag="ht")
                            nc.tensor.transpose(ht[:], h_sb[:, ci, :],
                                                ident_f[:])
                            ht_sb = spool.tile([F, P], f32, tag="hts")
                            nc.vector.tensor_copy(ht_sb[:], ht[:])
                            nc.tensor.matmul(
                                out=msd[:, ci * 66:(ci + 1) * 66],
                                lhsT=ht_sb[:], rhs=rhs_sb[l][:],
                                start=True, stop=True)
                        msd3 = msd[:].rearrange("p (c w) -> p c w", c=GROUP)
                        nc.vector.tensor_copy(stage_nxt[:, sl(g), 0:F],
                                              msd3[:, :, 0:F])
                        nc.vector.tensor_copy(
                            stage_nxt_f32[:, sl(g), HS_F32COL:HS_F32COL + 1],
                            msd3[:, :, F:F + 1])
                        nc.vector.tensor_copy(hs_nxt[:, sl(g)], msd3[:, :, F])
                        nc.vector.tensor_copy(hd_nxt[:, sl(g)],
                                              msd3[:, :, F + 1])
                    else:
                        sq = spool.tile([P, GROUP, F], f32, tag="h")
                        nc.vector.tensor_tensor(out=sq[:], in0=zb[:],
                                                in1=zb[:], op=ALU.mult)
                        n2 = spool.tile([P, GROUP], f32, tag="den")
                        nc.vector.reduce_sum(n2[:], sq[:],
                                             axis=mybir.AxisListType.X)
                        nrm = spool.tile([P, GROUP], f32, tag="rec")
                        nc.scalar.activation(nrm[:], n2[:], ACT.Sqrt)
                        nc.vector.tensor_scalar_max(nrm[:], nrm[:], 1e-12)
                        rinv = spool.tile([P, GROUP], f32, tag="nrmi")
                        nc.vector.reciprocal(rinv[:], nrm[:])
                        ob = spool.tile([P, GROUP, F], f32, tag="ob")
                        nc.vector.tensor_tensor(
                            out=ob[:], in0=zb[:],
                            in1=rinv[:][:, :, None].to_broadcast(
                                [P, GROUP, F]),
                            op=ALU.mult)
                        nc.sync.dma_start(
                            out_t[g * GROUP * P:(g + 1) * GROUP * P, :]
                            .rearrange("(c p) w -> p c w", p=P),
                            ob[:])

                    while (not last and cc_done < len(cc_ranges)
                           and g >= min(cc_ranges[cc_done][1] - 1 + CC_DELAY,
                                        NG - 1)):
                        publish(l + 1, stage_nxt, stage_nxt_f32, cc_done,
                                cc_ranges)
                        cc_done += 1

    nc.compile()
    return nc


# ======================= driver =======================

def kernel(x, src, dst, lin_w, lin_b, w1, a1s, a1d, b1, w2, a2s, a2d, b2,
           w3, a3s, a3d, b3):
    if "built" not in _cache:
        meta = _preprocess(src, dst)
        nc = _build_nc(meta)
        _cache["built"] = (meta, nc)
    meta, nc = _cache["built"]

    x = np.asarray(x, np.float32)
    lin_w = np.asarray(lin_w, np.float32)
    lin_b = np.asarray(lin_b, np.float32)
    ws = {1: np.asarray(w1, np.float32), 2: np.asarray(w2, np.float32),
          3: np.asarray(w3, np.float32)}
    avs = {1: (np.asarray(a1s, np.float32), np.asarray(a1d, np.float32)),
           2: (np.asarray(a2s, np.float32), np.asarray(a2d, np.float32)),
           3: (np.asarray(a3s, np.float32), np.asarray(a3d, np.float32))}
    bs = {1: np.asarray(b1, np.float32), 2: np.asarray(b2, np.float32),
          3: np.asarray(b3, np.float32)}

    node_at = meta["node_at"]
    newid = meta["newid"]
    gidx = meta["gidx"]

    # layer-1 message fold: m1 = (x@lin_w + lin_b) @ w1; s/d scores
    W1 = lin_w @ ws[1]                         # [256, 64]
    prew = np.concatenate(
        [W1, (W1 @ avs[1][0])[:, None], (W1 @ avs[1][1])[:, None]],
        axis=1).astype(np.float32)
    pb = lin_b @ ws[1]
    preb_row = np.concatenate([pb, [pb @ avs[1][0]], [pb @ avs[1][1]]])
    preb = np.tile(preb_row[None, :].astype(np.float32), (P, 1))
    ins_shared = {"prew": prew, "preb": preb, "expand": meta["expand"],
                  "padfix": np.frombuffer(np.float32(-1e5).tobytes(),
                                          dtype=ml_dtypes.bfloat16
                                          ).reshape(1, 2).copy()}
    for i, l in enumerate((2, 3)):
        wl = ws[l]
        ins_shared[f"rhs{i + 1}"] = np.concatenate(
            [wl, (wl @ avs[l][0])[:, None], (wl @ avs[l][1])[:, None]],
            axis=1).astype(np.float32)
    for l in (1, 2, 3):
        ins_shared[f"b{l}"] = np.tile(bs[l][None, :], (P, 1)).astype(np.float32)

    in_maps = []
    for k in range(NCORE):
        nodes = node_at[k].reshape(-1)
        xk = np.zeros((SPC, IND), np.float32)
        valid = nodes >= 0
        xk[valid] = x[nodes[valid]]
        m = dict(ins_shared)
        m["xT"] = np.ascontiguousarray(xk.T).astype(ml_dtypes.bfloat16)
        m["gidx"] = np.ascontiguousarray(gidx[k])
        in_maps.append(m)

    from concourse.bass_utils import run_bass_kernel_spmd
    import os
    trace = bool(os.environ.get("BASS_TRACE"))
    res = run_bass_kernel_spmd(nc, in_maps, list(range(NCORE)), trace=trace)
    if trace:
        globals()["LAST_EXEC_NS"] = res.exec_time_ns
        globals()["LAST_TRACE"] = (res.instructions_and_trace or (None, None))[1]
        globals()["LAST_PROFILE_JSON"] = res.profile_json

    out_new = np.concatenate([res.results[k]["out"] for k in range(NCORE)],
                             axis=0)
    return out_new[newid].astype(np.float32)
